# revision 4
# baseline (speedup 1.0000x reference)
"""DialogSeqAttnMatch Trainium2 kernel (8-core SPMD, L1-sharded), v2.

Math (reference):
  dlg   = concat(xq, xa) reshaped (B*M, H); M = LQ+LA
  x_proj = relu(xd @ W.T + b);  y_proj = relu(dlg @ W.T + b)
  scores[b,l,k] = x_proj[b,l] . y_proj[k]  masked (causal: ts(k) >= b, padding)
                  + rw0*|b - ts(k)|  (row 0 zeroed)
  out = softmax_k(scores) @ dlg   (row 0 of alpha zeroed -> out[0] = 0)

Key simplifications (carried over from v1):
  - In the causally valid region ts(k) < b, |b-ts| separates; the row factor
    e^{rw0 b} cancels in softmax and the column factor phi_k is folded into
    the value rows on the host.
  - Padding: the phi-scaled value rows AND the ones-column are zeroed on the
    host, so masked tokens drop out of numerator and denominator.
  - zero_first: host writes out[0] = 0.

v2 structure (per core: l-slice of 64 rows x 32 batches = 2048 l-cols):
  - 16 l-tiles of 128 l-cols (2 batches); k-chunks of 128 (2 timesteps).
    Jobs (t, c) for c <= t: 136 jobs, packed 8-per-supertile into
    [128, 1024] f32 PSUM tiles; ONE exp instruction per supertile.
  - All matmuls bf16 (1 cyc/col at any width).
  - Causal diagonal (c == t): partial matmuls touch only the valid
    (even-ts k) x (odd batch) region; dead PSUM/pt regions are never read.
    A 1-col "stopper" matmul closes the even-batch PSUM accumulation group.
  - A few supertiles' exp run on DVE via the Schraudolph bit trick
    (i16 = A*s + B bitcast to bf16) to unload ACT, the bottleneck engine.
  - Exp activation table preloaded by a dummy exp before the DMAs land.
"""
import os
import sys

sys.path.insert(0, "/opt/trn_rl_repo")

import numpy as np
import ml_dtypes

import concourse.bass as bass
import concourse.tile as tile
import concourse.mybir as mybir
from concourse import bacc
from concourse.bass_utils import run_bass_kernel_spmd

F32 = mybir.dt.float32
BF16 = mybir.dt.bfloat16
I16 = mybir.dt.int16
FP16 = mybir.dt.float16

B, L1, LQ, LA, H = 32, 512, 32, 32, 128
M = LQ + LA              # 64 tokens per timestep
K = B * M                # 2048 flattened history
NCORES = 8
LC = L1 // NCORES        # 64 l-rows per core
NT = 16                  # l-tiles (2 batches = 128 l-cols each)
S0 = 40.0                # exp shift (scores >= 0, max ~55)
T0 = 16.0                # phi centering

JOBS = [(t, c) for t in range(NT) for c in range(t + 1)]   # 136
NST = len(JOBS) // 8     # 17 supertiles of 8 jobs

# Supertiles whose exp runs on DVE (Schraudolph approx) instead of ACT.
SCH_STS = frozenset({9, 13})
SCH_A = 184.6650390625           # 128 * log2(e)
SCH_B = 16248.6 - SCH_A * S0 + 0.5   # +0.5: interp converts with trunc

# DMA pieces (cols) per input tensor, issued in consumption order.
Y_PIECES = [(0, 256), (256, 1024), (1024, 2048)]
X_PIECES = [(0, 256), (256, 1024), (1024, 2048)]
V_PIECES = [(0, 8 * 129), (8 * 129, 16 * 129)]

# Linear-projection pieces (y/x interleaved pairs sharing one PSUM tile).
PROJ_PIECES = [(0, 256), (256, 768), (768, 1280), (1280, 1792), (1792, 2048)]

_NC_CACHE = None


def _build():
    nc = bacc.Bacc("TRN2", target_bir_lowering=False, debug=False)

    inp_wb = nc.dram_tensor("inp_wb", [H, 130], FP16, kind="ExternalInput").ap()
    inp_y = nc.dram_tensor("inp_y", [H, K], FP16, kind="ExternalInput").ap()
    inp_x = nc.dram_tensor("inp_x", [H, B * LC], FP16, kind="ExternalInput").ap()
    inp_v = nc.dram_tensor("inp_v", [128, 16 * 129], FP16, kind="ExternalInput").ap()

    out = nc.dram_tensor("out", [B, LC, H], F32, kind="ExternalOutput").ap()
    out_flat = out.rearrange("b l d -> (b l) d")  # (2048, 128)

    with tile.TileContext(nc) as tc:
        with tc.tile_pool(name="const", bufs=1) as cpool, \
             tc.tile_pool(name="pt", bufs=3) as ptpool, \
             tc.tile_pool(name="osb", bufs=2) as osbpool, \
             tc.tile_pool(name="rcp", bufs=2) as rcppool, \
             tc.tile_pool(name="ps_big", bufs=3, space="PSUM") as psb, \
             tc.tile_pool(name="ps_out", bufs=2, space="PSUM") as pso:

            negs0 = cpool.tile([128, 1], F32)
            nc.vector.memset(negs0[:], -S0)
            # dummy exp: forces the ACT Exp table load during the DMA head
            scratch = cpool.tile([128, 1], BF16)
            nc.scalar.activation(scratch[:], negs0[:],
                                 mybir.ActivationFunctionType.Exp,
                                 bias=negs0[:], scale=1.0)

            wb_sb = cpool.tile([H, 130], FP16)
            wt_sb = wb_sb[:, 0:128]
            bcol_sb = wb_sb[:, 128:130].bitcast(F32)
            ysb = cpool.tile([H, K], FP16)
            xsb = cpool.tile([H, B * LC], FP16)
            vsb = cpool.tile([128, 16 * 129], FP16)

            nc.sync.dma_start(wb_sb[:], inp_wb)
            nc.sync.dma_start(ysb[:, 0:256], inp_y[:, 0:256])
            nc.sync.dma_start(xsb[:, 0:256], inp_x[:, 0:256])
            nc.sync.dma_start(ysb[:, 256:1024], inp_y[:, 256:1024])
            nc.sync.dma_start(xsb[:, 256:1024], inp_x[:, 256:1024])
            nc.sync.dma_start(vsb[:, 0:1032], inp_v[:, 0:1032])
            nc.sync.dma_start(ysb[:, 1024:2048], inp_y[:, 1024:2048])
            nc.sync.dma_start(xsb[:, 1024:2048], inp_x[:, 1024:2048])
            nc.sync.dma_start(vsb[:, 1032:2064], inp_v[:, 1032:2064])

            yproj = cpool.tile([H, K], FP16)
            xproj = cpool.tile([H, B * LC], FP16)

            next_piece = {"y": 0, "x": 0}

            def emit_proj_piece(i):
                lo, hi = PROJ_PIECES[i]
                n = hi - lo
                ps = psb.tile([128, 1024], F32, tag="scps", name=f"psproj{i}")
                nc.tensor.matmul(ps[:, 0:n], wt_sb, ysb[:, lo:hi],
                                 start=True, stop=True)
                nc.tensor.matmul(ps[:, 512:512 + n], wt_sb, xsb[:, lo:hi],
                                 start=True, stop=True)
                nc.vector.tensor_scalar(yproj[:, lo:hi], ps[:, 0:n],
                                        bcol_sb, 0.0,
                                        op0=mybir.AluOpType.add,
                                        op1=mybir.AluOpType.max)
                nc.vector.tensor_scalar(xproj[:, lo:hi], ps[:, 512:512 + n],
                                        bcol_sb, 0.0,
                                        op0=mybir.AluOpType.add,
                                        op1=mybir.AluOpType.max)

            def need_proj(upto):
                # proj pieces cover y and x symmetrically; jobs need
                # yproj[<128(c+1)] and xproj[<128(t+1)], and c <= t.
                while next_piece["y"] < len(PROJ_PIECES) and \
                        PROJ_PIECES[next_piece["y"]][0] < upto:
                    emit_proj_piece(next_piece["y"])
                    next_piece["y"] += 1

            ps_tiles = {}
            pso_state = {}
            osb_state = {}

            def emit_scores_st(st):
                jobs_st = JOBS[8 * st:8 * st + 8]
                need_proj(128 * (max(t for t, c in jobs_st) + 1))
                ps = psb.tile([128, 1024], F32, tag="scps")
                ps_tiles[st] = ps
                for idx, (t, c) in enumerate(jobs_st):
                    off = idx * 128
                    if c < t:
                        nc.tensor.matmul(ps[:, off:off + 128],
                                         yproj[:, 128 * c:128 * c + 128],
                                         xproj[:, 128 * t:128 * t + 128],
                                         start=True, stop=True)
                    else:  # diagonal: only even-ts k x odd batch is valid
                        nc.tensor.matmul(ps[0:64, off + 64:off + 128],
                                         yproj[:, 128 * c:128 * c + 64],
                                         xproj[:, 128 * t + 64:128 * t + 128],
                                         start=True, stop=True)

            def emit_exp(st, pt, ps, lo, hi):
                if st in SCH_STS:
                    nc.vector.tensor_scalar(pt[:].bitcast(I16)[:, lo:hi],
                                            ps[:, lo:hi], SCH_A, SCH_B,
                                            op0=mybir.AluOpType.mult,
                                            op1=mybir.AluOpType.add)
                else:
                    nc.scalar.activation(pt[:, lo:hi], ps[:, lo:hi],
                                         mybir.ActivationFunctionType.Exp,
                                         bias=negs0[:], scale=1.0)

            def emit_out_job(st, pt, idx, t, c):
                off = idx * 128
                vchunk = vsb[:, 129 * c:129 * c + 129]
                if c == 0:
                    pso_state[t] = pso.tile([128, 512], F32, tag="psout",
                                            name=f"pso{t}")
                ps_o = pso_state[t]
                if c < t:
                    nc.tensor.matmul(ps_o[:, 0:129], pt[:, off:off + 128],
                                     vchunk, start=(c == 0), stop=False)
                else:  # diagonal
                    nc.tensor.matmul(ps_o[64:128, 0:129],
                                     pt[0:64, off + 64:off + 128],
                                     vchunk[0:64, :],
                                     start=(t == 0), stop=True)
                    if t > 0:
                        # close the even-batch (partitions 0:64) psum group
                        nc.tensor.matmul(ps_o[0:64, 129:130],
                                         pt[0:64, off:off + 64],
                                         vchunk[0:64, 0:1],
                                         start=False, stop=True)
                    emit_normalize(t)

            def emit_process_st(st):
                jobs_st = JOBS[8 * st:8 * st + 8]
                ps = ps_tiles.pop(st)
                pt = ptpool.tile([128, 1024], BF16, tag="pt")
                if st == NST - 1:
                    # split the last exp so the drain chain is short
                    emit_exp(st, pt, ps, 0, 896)
                    for idx, (t, c) in enumerate(jobs_st[:7]):
                        emit_out_job(st, pt, idx, t, c)
                    emit_exp(st, pt, ps, 896, 1024)
                    t, c = jobs_st[7]
                    emit_out_job(st, pt, 7, t, c)
                else:
                    emit_exp(st, pt, ps, 0, 1024)
                    for idx, (t, c) in enumerate(jobs_st):
                        emit_out_job(st, pt, idx, t, c)

            def emit_normalize(t):
                ps_o = pso_state.pop(t)
                pair, slot = divmod(t, 2)
                solo = t >= 14
                if solo:
                    osb = osbpool.tile([128, 128], F32, tag="osbs",
                                       name=f"osbs{t}")
                elif slot == 0:
                    osb_state[pair] = osbpool.tile([128, 256], F32, tag="osb",
                                                   name=f"osb{pair}")
                    osb = osb_state[pair]
                else:
                    osb = osb_state[pair]
                recip = rcppool.tile([128, 1], F32, tag="recip")
                nc.vector.reciprocal(recip[:], ps_o[:, 128:129])
                dst = osb[:] if solo else osb[:, 128 * slot:128 * slot + 128]
                nc.vector.tensor_scalar_mul(dst, ps_o[:, 0:128], recip[:])
                if solo:
                    nc.sync.dma_start(out_flat[128 * t:128 * t + 128], osb[:])
                elif slot == 1:
                    osb_state.pop(pair)
                    dsl = out_flat[256 * pair:256 * pair + 256].rearrange(
                        "(h p) d -> p h d", h=2)
                    nc.sync.dma_start(dsl,
                                      osb[:].rearrange("p (h d) -> p h d", h=2))

            # software pipeline: scores run LOOKAHEAD supertiles ahead
            LOOKAHEAD = 2
            for st in range(NST + LOOKAHEAD):
                if st < NST:
                    emit_scores_st(st)
                j = st - LOOKAHEAD
                if j >= 0:
                    emit_process_st(j)

    nc.compile()
    return nc


def _get_nc():
    global _NC_CACHE
    if _NC_CACHE is None:
        _NC_CACHE = _build()
    return _NC_CACHE


LAST_RESULTS = None  # BassKernelResults of the most recent run (for test harness)


def kernel(xd_emb, xq_emb, xa_emb, W, b, recency_weight, xq_mask, xa_mask,
           _trace=False):
    xd_emb = np.asarray(xd_emb, np.float32)
    xq_emb = np.asarray(xq_emb, np.float32)
    xa_emb = np.asarray(xa_emb, np.float32)
    W = np.asarray(W, np.float32)
    b = np.asarray(b, np.float32)
    rw0 = float(np.asarray(recency_weight).reshape(-1)[0])
    pad = np.concatenate([np.asarray(xq_mask), np.asarray(xa_mask)],
                         axis=1).reshape(K)

    dlg = np.concatenate([xq_emb, xa_emb], axis=1).reshape(K, H)
    ts = (np.arange(K) // M).astype(np.float64)
    phi = np.exp(-rw0 * (ts - T0))
    dlg_aug = np.concatenate([dlg.astype(np.float64), np.ones((K, 1))], axis=1)
    dlg_aug *= phi[:, None]
    dlg_aug[pad] = 0.0
    dlga_bf = dlg_aug.astype(np.float16)
    inp_v = np.ascontiguousarray(
        dlga_bf.reshape(16, 128, 129).transpose(1, 0, 2).reshape(128, 16 * 129))

    inp_wb = np.empty((H, 130), np.float16)
    inp_wb[:, 0:128] = W.T.astype(np.float16)
    inp_wb[:, 128:130] = np.ascontiguousarray(
        b.reshape(H, 1).astype(np.float32)).view(np.uint16).view(
            np.float16)

    inp_y = np.ascontiguousarray(dlg.T).astype(np.float16)

    xdT = xd_emb.transpose(2, 0, 1)  # (H, B, L1)
    in_maps = []
    for c in range(NCORES):
        xdT_c = xdT[:, :, c * LC:(c + 1) * LC].reshape(H, B * LC)
        in_maps.append({
            "inp_wb": inp_wb,
            "inp_y": inp_y,
            "inp_x": np.ascontiguousarray(xdT_c).astype(np.float16),
            "inp_v": inp_v,
        })

    nc = _get_nc()
    try:
        res = run_bass_kernel_spmd(nc, in_maps, list(range(NCORES)),
                                   trace=_trace)
    except ModuleNotFoundError:
        # The axon NTFF-profile hook is absent in this container; if an
        # ambient BASS_TRACE forced the trace path, retry without it.
        os.environ["BASS_NEVER_TRACE"] = "1"
        res = run_bass_kernel_spmd(nc, in_maps, list(range(NCORES)))
    global LAST_RESULTS
    LAST_RESULTS = res
    parts = [res.results[c]["out"] for c in range(NCORES)]
    full = np.concatenate(parts, axis=1)  # (32, 512, 128)
    full[0] = 0.0
    return np.ascontiguousarray(full, dtype=np.float32)


# revision 6
# speedup vs baseline: 1.0738x; 1.0738x over previous
"""DialogSeqAttnMatch Trainium2 kernel (8-core SPMD, L1-sharded), v2.

Math (reference):
  dlg   = concat(xq, xa) reshaped (B*M, H); M = LQ+LA
  x_proj = relu(xd @ W.T + b);  y_proj = relu(dlg @ W.T + b)
  scores[b,l,k] = x_proj[b,l] . y_proj[k]  masked (causal: ts(k) >= b, padding)
                  + rw0*|b - ts(k)|  (row 0 zeroed)
  out = softmax_k(scores) @ dlg   (row 0 of alpha zeroed -> out[0] = 0)

Key simplifications (carried over from v1):
  - In the causally valid region ts(k) < b, |b-ts| separates; the row factor
    e^{rw0 b} cancels in softmax and the column factor phi_k is folded into
    the value rows on the host.
  - Padding: the phi-scaled value rows AND the ones-column are zeroed on the
    host, so masked tokens drop out of numerator and denominator.
  - zero_first: host writes out[0] = 0.

v2 structure (per core: l-slice of 64 rows x 32 batches = 2048 l-cols):
  - 16 l-tiles of 128 l-cols (2 batches); k-chunks of 128 (2 timesteps).
    Jobs (t, c) for c <= t: 136 jobs, packed 8-per-supertile into
    [128, 1024] f32 PSUM tiles; ONE exp instruction per supertile.
  - All matmuls bf16 (1 cyc/col at any width).
  - Causal diagonal (c == t): partial matmuls touch only the valid
    (even-ts k) x (odd batch) region; dead PSUM/pt regions are never read.
    A 1-col "stopper" matmul closes the even-batch PSUM accumulation group.
  - A few supertiles' exp run on DVE via the Schraudolph bit trick
    (i16 = A*s + B bitcast to bf16) to unload ACT, the bottleneck engine.
  - Exp activation table preloaded by a dummy exp before the DMAs land.
"""
import os
import sys

sys.path.insert(0, "/opt/trn_rl_repo")

import numpy as np
import ml_dtypes

import concourse.bass as bass
import concourse.tile as tile
import concourse.mybir as mybir
from concourse import bacc
from concourse.bass_utils import run_bass_kernel_spmd

F32 = mybir.dt.float32
BF16 = mybir.dt.bfloat16
I16 = mybir.dt.int16
FP16 = mybir.dt.float16

B, L1, LQ, LA, H = 32, 512, 32, 32, 128
M = LQ + LA              # 64 tokens per timestep
K = B * M                # 2048 flattened history
NCORES = 8
LC = L1 // NCORES        # 64 l-rows per core
NT = 16                  # l-tiles (2 batches = 128 l-cols each)
S0 = 40.0                # exp shift (scores >= 0, max ~55)
T0 = 16.0                # phi centering

JOBS = [(t, c) for t in range(NT) for c in range(t + 1)]   # 136
NST = len(JOBS) // 8     # 17 supertiles of 8 jobs

# Supertiles whose exp runs on DVE (Schraudolph approx) instead of ACT.
SCH_STS = frozenset({9, 13})
SCH_A = 184.6650390625           # 128 * log2(e)
SCH_B = 16248.6 - SCH_A * S0 + 0.5   # +0.5: interp converts with trunc

# DMA pieces (cols) per input tensor, issued in consumption order.
Y_PIECES = [(0, 256), (256, 1024), (1024, 2048)]
X_PIECES = [(0, 256), (256, 1024), (1024, 2048)]
V_PIECES = [(0, 8 * 129), (8 * 129, 16 * 129)]

# Packed-input segment layout (cols of inp_all), consumption order.
SEGS = [("wb", 130), ("y0", 256), ("x0", 256), ("y1", 768), ("x1", 768),
        ("v0", 1032), ("y2", 1024), ("x2", 1024), ("v1", 1032)]
SEG_OFF = {}
_o = 0
for _n, _w in SEGS:
    SEG_OFF[_n] = _o
    _o += _w
NSEG_TOT = _o
# y/x segment boundaries in logical cols -> (segment, base logical col)
YX_SEGS = [(0, 256, "y0", "x0"), (256, 1024, "y1", "x1"),
           (1024, 2048, "y2", "x2")]
# DMA spans over inp_all (consumption order)
DMA_SPANS = [(0, SEG_OFF["y1"]),                     # wb + y0 + x0
             (SEG_OFF["y1"], SEG_OFF["v0"]),         # y1 + x1
             (SEG_OFF["v0"], SEG_OFF["y2"]),         # v0
             (SEG_OFF["y2"], SEG_OFF["v1"]),         # y2 + x2
             (SEG_OFF["v1"], NSEG_TOT)]              # v1
# Linear-projection pieces = the y/x segments.
PROJ_PIECES = [(0, 256), (256, 1024), (1024, 2048)]

_NC_CACHE = None


def _build():
    nc = bacc.Bacc("TRN2", target_bir_lowering=False, debug=False)

    # single packed input tensor, segments in consumption order:
    # [wb 130 | y0 256 | x0 256 | y1 768 | x1 768 | v0 1032 | y2 1024 |
    #  x2 1024 | v1 1032]  (all fp16; wb = W.T cols + f32 bias as 2 cols)
    inp_all = nc.dram_tensor("inp_all", [128, NSEG_TOT], FP16,
                             kind="ExternalInput").ap()

    out = nc.dram_tensor("out", [B, LC, H], F32, kind="ExternalOutput").ap()
    out_flat = out.rearrange("b l d -> (b l) d")  # (2048, 128)

    with tile.TileContext(nc) as tc:
        with tc.tile_pool(name="const", bufs=1) as cpool, \
             tc.tile_pool(name="pt", bufs=3) as ptpool, \
             tc.tile_pool(name="osb", bufs=2) as osbpool, \
             tc.tile_pool(name="rcp", bufs=2) as rcppool, \
             tc.tile_pool(name="ps_big", bufs=3, space="PSUM") as psb, \
             tc.tile_pool(name="ps_out", bufs=2, space="PSUM") as pso:

            negs0 = cpool.tile([128, 1], F32)
            nc.vector.memset(negs0[:], -S0)
            # dummy exp: forces the ACT Exp table load during the DMA head
            scratch = cpool.tile([128, 1], BF16)
            nc.scalar.activation(scratch[:], negs0[:],
                                 mybir.ActivationFunctionType.Exp,
                                 bias=negs0[:], scale=1.0)

            mega = cpool.tile([128, NSEG_TOT], FP16)
            wt_sb = mega[:, 0:128]
            bcol_sb = mega[:, 128:130].bitcast(F32)

            def yseg(lo, hi):
                for a, b2, yn, xn in YX_SEGS:
                    if lo >= a and hi <= b2:
                        o = SEG_OFF[yn] + (lo - a)
                        return mega[:, o:o + (hi - lo)]
                raise AssertionError((lo, hi))

            def xseg(lo, hi):
                for a, b2, yn, xn in YX_SEGS:
                    if lo >= a and hi <= b2:
                        o = SEG_OFF[xn] + (lo - a)
                        return mega[:, o:o + (hi - lo)]
                raise AssertionError((lo, hi))

            def vchunk_ap(c):
                seg = "v0" if c < 8 else "v1"
                o = SEG_OFF[seg] + 129 * (c % 8)
                return mega[:, o:o + 129]

            for lo, hi in DMA_SPANS:
                nc.sync.dma_start(mega[:, lo:hi], inp_all[:, lo:hi])

            yproj = cpool.tile([H, K], FP16)
            xproj = cpool.tile([H, B * LC], FP16)

            next_piece = {"y": 0, "x": 0}

            def emit_proj_piece(i):
                lo, hi = PROJ_PIECES[i]
                n = hi - lo
                for src_ap, dst in ((yseg(lo, hi), yproj), (xseg(lo, hi), xproj)):
                    ps = psb.tile([128, 1024], F32, tag="scps",
                                  name=f"psproj{i}{dst is xproj}")
                    for a in range(0, n, 512):  # moving free dim cap is 512
                        b3 = min(a + 512, n)
                        nc.tensor.matmul(ps[:, a:b3], wt_sb, src_ap[:, a:b3],
                                         start=True, stop=True)
                    nc.vector.tensor_scalar(dst[:, lo:hi], ps[:, 0:n],
                                            bcol_sb, 0.0,
                                            op0=mybir.AluOpType.add,
                                            op1=mybir.AluOpType.max)

            def need_proj(upto):
                # proj pieces cover y and x symmetrically; jobs need
                # yproj[<128(c+1)] and xproj[<128(t+1)], and c <= t.
                while next_piece["y"] < len(PROJ_PIECES) and \
                        PROJ_PIECES[next_piece["y"]][0] < upto:
                    emit_proj_piece(next_piece["y"])
                    next_piece["y"] += 1

            ps_tiles = {}
            pso_state = {}
            osb_state = {}

            def emit_scores_st(st):
                jobs_st = JOBS[8 * st:8 * st + 8]
                need_proj(128 * (max(t for t, c in jobs_st) + 1))
                ps = psb.tile([128, 1024], F32, tag="scps")
                ps_tiles[st] = ps
                for idx, (t, c) in enumerate(jobs_st):
                    off = idx * 128
                    if c < t:
                        nc.tensor.matmul(ps[:, off:off + 128],
                                         yproj[:, 128 * c:128 * c + 128],
                                         xproj[:, 128 * t:128 * t + 128],
                                         start=True, stop=True)
                    else:  # diagonal: only even-ts k x odd batch is valid
                        nc.tensor.matmul(ps[0:64, off + 64:off + 128],
                                         yproj[:, 128 * c:128 * c + 64],
                                         xproj[:, 128 * t + 64:128 * t + 128],
                                         start=True, stop=True)

            def emit_exp(st, pt, ps, lo, hi):
                if st in SCH_STS:
                    nc.vector.tensor_scalar(pt[:].bitcast(I16)[:, lo:hi],
                                            ps[:, lo:hi], SCH_A, SCH_B,
                                            op0=mybir.AluOpType.mult,
                                            op1=mybir.AluOpType.add)
                else:
                    nc.scalar.activation(pt[:, lo:hi], ps[:, lo:hi],
                                         mybir.ActivationFunctionType.Exp,
                                         bias=negs0[:], scale=1.0)

            def emit_out_job(st, pt, idx, t, c):
                off = idx * 128
                vchunk = vchunk_ap(c)
                if c == 0:
                    pso_state[t] = pso.tile([128, 512], F32, tag="psout",
                                            name=f"pso{t}")
                ps_o = pso_state[t]
                if c < t:
                    nc.tensor.matmul(ps_o[:, 0:129], pt[:, off:off + 128],
                                     vchunk, start=(c == 0), stop=False)
                else:  # diagonal
                    nc.tensor.matmul(ps_o[64:128, 0:129],
                                     pt[0:64, off + 64:off + 128],
                                     vchunk[0:64, :],
                                     start=(t == 0), stop=True)
                    if t > 0:
                        # close the even-batch (partitions 0:64) psum group
                        nc.tensor.matmul(ps_o[0:64, 129:130],
                                         pt[0:64, off:off + 64],
                                         vchunk[0:64, 0:1],
                                         start=False, stop=True)
                    emit_normalize(t)

            def emit_process_st(st):
                jobs_st = JOBS[8 * st:8 * st + 8]
                ps = ps_tiles.pop(st)
                pt = ptpool.tile([128, 1024], BF16, tag="pt")
                if st == NST - 1:
                    # split the last exp so the drain chain is short
                    emit_exp(st, pt, ps, 0, 896)
                    for idx, (t, c) in enumerate(jobs_st[:7]):
                        emit_out_job(st, pt, idx, t, c)
                    emit_exp(st, pt, ps, 896, 1024)
                    t, c = jobs_st[7]
                    emit_out_job(st, pt, 7, t, c)
                else:
                    emit_exp(st, pt, ps, 0, 1024)
                    for idx, (t, c) in enumerate(jobs_st):
                        emit_out_job(st, pt, idx, t, c)

            def emit_normalize(t):
                ps_o = pso_state.pop(t)
                pair, slot = divmod(t, 2)
                solo = t >= 14
                if solo:
                    osb = osbpool.tile([128, 128], F32, tag="osbs",
                                       name=f"osbs{t}")
                elif slot == 0:
                    osb_state[pair] = osbpool.tile([128, 256], F32, tag="osb",
                                                   name=f"osb{pair}")
                    osb = osb_state[pair]
                else:
                    osb = osb_state[pair]
                recip = rcppool.tile([128, 1], F32, tag="recip")
                nc.vector.reciprocal(recip[:], ps_o[:, 128:129])
                dst = osb[:] if solo else osb[:, 128 * slot:128 * slot + 128]
                nc.vector.tensor_scalar_mul(dst, ps_o[:, 0:128], recip[:])
                if solo:
                    nc.sync.dma_start(out_flat[128 * t:128 * t + 128], osb[:])
                elif slot == 1:
                    osb_state.pop(pair)
                    dsl = out_flat[256 * pair:256 * pair + 256].rearrange(
                        "(h p) d -> p h d", h=2)
                    nc.sync.dma_start(dsl,
                                      osb[:].rearrange("p (h d) -> p h d", h=2))

            # software pipeline: scores run LOOKAHEAD supertiles ahead
            LOOKAHEAD = 2
            for st in range(NST + LOOKAHEAD):
                if st < NST:
                    emit_scores_st(st)
                j = st - LOOKAHEAD
                if j >= 0:
                    emit_process_st(j)

    nc.compile()
    return nc


def _get_nc():
    global _NC_CACHE
    if _NC_CACHE is None:
        _NC_CACHE = _build()
    return _NC_CACHE


LAST_RESULTS = None  # BassKernelResults of the most recent run (for test harness)


def kernel(xd_emb, xq_emb, xa_emb, W, b, recency_weight, xq_mask, xa_mask,
           _trace=False):
    xd_emb = np.asarray(xd_emb, np.float32)
    xq_emb = np.asarray(xq_emb, np.float32)
    xa_emb = np.asarray(xa_emb, np.float32)
    W = np.asarray(W, np.float32)
    b = np.asarray(b, np.float32)
    rw0 = float(np.asarray(recency_weight).reshape(-1)[0])
    pad = np.concatenate([np.asarray(xq_mask), np.asarray(xa_mask)],
                         axis=1).reshape(K)

    dlg = np.concatenate([xq_emb, xa_emb], axis=1).reshape(K, H)
    ts = (np.arange(K) // M).astype(np.float64)
    phi = np.exp(-rw0 * (ts - T0))
    dlg_aug = np.concatenate([dlg.astype(np.float64), np.ones((K, 1))], axis=1)
    dlg_aug *= phi[:, None]
    dlg_aug[pad] = 0.0
    dlga_bf = dlg_aug.astype(np.float16)
    inp_v = np.ascontiguousarray(
        dlga_bf.reshape(16, 128, 129).transpose(1, 0, 2).reshape(128, 16 * 129))

    yT = np.ascontiguousarray(dlg.T).astype(np.float16)

    xdT = xd_emb.transpose(2, 0, 1)  # (H, B, L1)
    in_maps = []
    for c in range(NCORES):
        xdT_c = np.ascontiguousarray(
            xdT[:, :, c * LC:(c + 1) * LC].reshape(H, B * LC)).astype(
                np.float16)
        inp_all = np.empty((128, NSEG_TOT), np.float16)
        inp_all[:, 0:128] = W.T.astype(np.float16)
        inp_all[:, 128:130] = np.ascontiguousarray(
            b.reshape(H, 1).astype(np.float32)).view(np.uint16).view(
                np.float16)
        for a, b2, yn, xn in YX_SEGS:
            inp_all[:, SEG_OFF[yn]:SEG_OFF[yn] + b2 - a] = yT[:, a:b2]
            inp_all[:, SEG_OFF[xn]:SEG_OFF[xn] + b2 - a] = xdT_c[:, a:b2]
        inp_all[:, SEG_OFF["v0"]:SEG_OFF["v0"] + 1032] = inp_v[:, 0:1032]
        inp_all[:, SEG_OFF["v1"]:SEG_OFF["v1"] + 1032] = inp_v[:, 1032:2064]
        in_maps.append({"inp_all": inp_all})

    nc = _get_nc()
    try:
        res = run_bass_kernel_spmd(nc, in_maps, list(range(NCORES)),
                                   trace=_trace)
    except ModuleNotFoundError:
        # The axon NTFF-profile hook is absent in this container; if an
        # ambient BASS_TRACE forced the trace path, retry without it.
        os.environ["BASS_NEVER_TRACE"] = "1"
        res = run_bass_kernel_spmd(nc, in_maps, list(range(NCORES)))
    global LAST_RESULTS
    LAST_RESULTS = res
    parts = [res.results[c]["out"] for c in range(NCORES)]
    full = np.concatenate(parts, axis=1)  # (32, 512, 128)
    full[0] = 0.0
    return np.ascontiguousarray(full, dtype=np.float32)
